# revision 3
# baseline (speedup 1.0000x reference)
"""SwiGLU-projected causal MHA (B=4, S=2048, D=1024, H=16) on 8 TRN2 NeuronCores.

Sharding: core c -> (batch b = c//2, head-group g = c%2).  Each core computes
the SwiGLU Q/K/V projections for its 512 output channels (= 8 heads) of its
batch, runs causal attention for those heads, and produces a partial output
projection (contraction over its 512 channels).  The host sums the two
partials per batch and adds the output bias.

Device layout (per core):
  QT/KT [128p, 4j, 2048n]   channels on partitions (local ch = j*128 + p),
                            seq on free.  Head hl -> chunk hl//2, partition
                            base 64*(hl%2); head pairs run concurrently on
                            the PE via disjoint row groups (K=64 matmuls at
                            base 0 / base 64).
  V     [128p, 16nt, 8hl, 65]  seq on partitions (n = nt*128+p); per head 64
                            channels plus a ones column, so the AV matmul
                            emits the softmax denominator as output row 64.
  Scores are computed transposed, S^T [k-part, q-free], exp'd on the scalar
  engine straight out of PSUM (no max subtraction -- logits are bounded),
  masked multiplicatively on diagonal blocks only, and consumed as the
  moving operand of the AV matmul.  All matmuls run in float32r (full-rate
  fp32 streaming; measured ~1.6e-4 relative error on HW).
"""
import sys

sys.path.insert(0, "/opt/trn_rl_repo")
import numpy as np

import concourse.bacc as bacc
import concourse.tile as tile
import concourse.mybir as mybir

B, S, D = 4, 2048, 1024
H, DK = 16, 64
NCORES = 8
GCH = 512          # channels per core (8 heads)
NT = S // 128      # 16 seq chunks
F32 = mybir.dt.float32
F32R = mybir.dt.float32r
ACTF = mybir.ActivationFunctionType
ALU = mybir.AluOpType

TRACE = False          # set by test.py for profiling runs
TRACE_CORES = None
LAST_RESULT = None     # BassKernelResults stash for test.py
USE_SILU = True        # HW has a Silu table; CoreSim only has Sigmoid


def build_program(mask_mode):
    """mask_mode: 'causal' (tril), 'full' (all ones), 'general' (arbitrary)."""
    nc = bacc.Bacc("TRN2", target_bir_lowering=False, debug=False)

    xT = {s: nc.dram_tensor(f"x{s}T", [D, S], F32R, kind="ExternalInput")
          for s in "qkv"}
    w1T = {s: nc.dram_tensor(f"w1T_{s}", [D, GCH], F32R, kind="ExternalInput")
           for s in "qkv"}
    w2T = {s: nc.dram_tensor(f"w2T_{s}", [D, GCH], F32R, kind="ExternalInput")
           for s in "qkv"}
    bias_d = {}
    for s in "qk":
        bias_d[f"b1_{s}"] = nc.dram_tensor(f"b1_{s}", [128, 4], F32,
                                           kind="ExternalInput")
        bias_d[f"b2_{s}"] = nc.dram_tensor(f"b2_{s}", [128, 4], F32,
                                           kind="ExternalInput")
    b1v_d = nc.dram_tensor("b1_v", [1, GCH], F32R, kind="ExternalInput")
    b2v_d = nc.dram_tensor("b2_v", [1, GCH], F32R, kind="ExternalInput")
    woT_d = nc.dram_tensor("woT", [128, 4, D], F32R, kind="ExternalInput")
    pat_d = m01T_d = None
    if mask_mode == "causal":
        pat_d = nc.dram_tensor("pat", [128, 4, 512], F32, kind="ExternalInput")
    elif mask_mode == "general":
        m01T_d = nc.dram_tensor("m01T", [S, S], F32, kind="ExternalInput")
    pout_d = nc.dram_tensor("pout", [S, D], F32, kind="ExternalOutput")

    def kc_count(qg):
        return 4 * qg + 4 if mask_mode == "causal" else NT

    with tile.TileContext(nc) as tc:
        with (
            tc.tile_pool(name="persist", bufs=1) as persist,
        ):
            qt_sb = persist.tile([128, 4, S], F32R, tag="qt")
            kt_sb = persist.tile([128, 4, S], F32R, tag="kt")
            v_sb = persist.tile([128, NT, 8, 65], F32R, tag="v")
            woT_sb = persist.tile([128, 4, D], F32R, tag="wo")
            nc.sync.dma_start(woT_sb[:], woT_d[:])
            onesf = persist.tile([1, 128], F32, tag="onesf")
            ones_r = persist.tile([1, 128], F32R, tag="ones_r")
            nc.any.memset(onesf[:], 1.0)
            nc.vector.tensor_copy(ones_r[:], onesf[:])
            onescol = persist.tile([128, 1], F32, tag="onescol")
            nc.any.memset(onescol[:], 1.0)
            nc.vector.tensor_copy(
                v_sb[:, :, :, 64:65],
                onescol[:, None, :].to_broadcast([128, NT, 8, 1]),
            )
            if mask_mode == "causal":
                pat_sb = persist.tile([128, 4, 512], F32, tag="pat")
                nc.sync.dma_start(pat_sb[:], pat_d[:])

            # ---------------- Phase A: SwiGLU projections ----------------
            with (
                tc.tile_pool(name="wpool", bufs=2) as wpool,
                tc.tile_pool(name="xpool", bufs=9) as xpool,
                tc.tile_pool(name="stage", bufs=4) as stage,
                tc.tile_pool(name="pps", bufs=6, space="PSUM") as pps,
            ):
                for s in "qkv":
                    w1sb = wpool.tile([128, 8, GCH], F32R, tag="w")
                    w2sb = wpool.tile([128, 8, GCH], F32R, tag="w")
                    nc.sync.dma_start(
                        w1sb[:], w1T[s][:].rearrange("(dc p) o -> p dc o", p=128)
                    )
                    nc.sync.dma_start(
                        w2sb[:], w2T[s][:].rearrange("(dc p) o -> p dc o", p=128)
                    )
                    if s != "v":
                        b1sb = persist.tile([128, 4], F32, tag=f"b1{s}")
                        b2sb = persist.tile([128, 4], F32, tag=f"b2{s}")
                        nc.sync.dma_start(b1sb[:], bias_d[f"b1_{s}"][:])
                        nc.sync.dma_start(b2sb[:], bias_d[f"b2_{s}"][:])
                    else:
                        b1vr = persist.tile([1, GCH], F32R, tag="b1v")
                        b2vr = persist.tile([1, GCH], F32R, tag="b2v")
                        nc.sync.dma_start(b1vr[:], b1v_d[:])
                        nc.sync.dma_start(b2vr[:], b2v_d[:])

                    for t in range(4):  # 512-wide seq tiles
                        xts = []
                        for dc in range(8):
                            xt = xpool.tile([128, 512], F32R, tag="xt")
                            nc.sync.dma_start(
                                xt[:],
                                xT[s][dc * 128:(dc + 1) * 128,
                                      t * 512:(t + 1) * 512],
                            )
                            xts.append(xt)
                        for jh in range(2):
                            ps1 = [pps.tile([128, 512], F32, tag="pp",
                                            name=f"ps1_{i}")
                                   for i in range(2)]
                            ps2 = [pps.tile([128, 512], F32, tag="pp",
                                            name=f"ps2_{i}")
                                   for i in range(2)]
                            for dc in range(8):
                                for jj in range(2):
                                    j = jh * 2 + jj
                                    if s == "v":
                                        # seq on partitions: lhsT = x chunk
                                        nc.tensor.matmul(
                                            ps1[jj][:],
                                            xts[dc][:, j * 128:(j + 1) * 128],
                                            w1sb[:, dc, :],
                                            start=(dc == 0), stop=False,
                                        )
                                        nc.tensor.matmul(
                                            ps2[jj][:],
                                            xts[dc][:, j * 128:(j + 1) * 128],
                                            w2sb[:, dc, :],
                                            start=(dc == 0), stop=False,
                                        )
                                    else:
                                        # channels on partitions: lhsT = w chunk
                                        nc.tensor.matmul(
                                            ps1[jj][:],
                                            w1sb[:, dc, j * 128:(j + 1) * 128],
                                            xts[dc][:],
                                            start=(dc == 0), stop=(dc == 7),
                                        )
                                        nc.tensor.matmul(
                                            ps2[jj][:],
                                            w2sb[:, dc, j * 128:(j + 1) * 128],
                                            xts[dc][:],
                                            start=(dc == 0), stop=(dc == 7),
                                        )
                            for jj in range(2):
                                j = jh * 2 + jj
                                act = stage.tile([128, 512], F32, tag="act")
                                if s == "v":
                                    # fold the biases into the accumulation
                                    # (they vary along the free/channel dim)
                                    nc.tensor.matmul(
                                        ps1[jj][:], ones_r[:], b1vr[:],
                                        start=False, stop=True,
                                    )
                                    nc.tensor.matmul(
                                        ps2[jj][:], ones_r[:], b2vr[:],
                                        start=False, stop=True,
                                    )
                                    if USE_SILU:
                                        nc.scalar.activation(
                                            act[:], ps1[jj][:], ACTF.Silu
                                        )
                                    else:
                                        nc.scalar.activation(
                                            act[:], ps1[jj][:], ACTF.Sigmoid
                                        )
                                        nc.vector.tensor_tensor(
                                            act[:], ps1[jj][:], act[:],
                                            ALU.mult,
                                        )
                                    nt_i = t * 4 + j
                                    nc.vector.tensor_tensor(
                                        v_sb[:, nt_i, :, 0:64],
                                        ps2[jj][:].rearrange(
                                            "p (h d) -> p h d", h=8
                                        ),
                                        act[:].rearrange(
                                            "p (h d) -> p h d", h=8
                                        ),
                                        ALU.mult,
                                    )
                                else:
                                    bias1 = b1sb[:, j:j + 1]
                                    bias2 = b2sb[:, j:j + 1]
                                    if USE_SILU:
                                        nc.scalar.activation(
                                            act[:], ps1[jj][:], ACTF.Silu,
                                            bias=bias1,
                                        )
                                    else:
                                        nc.scalar.activation(
                                            act[:], ps1[jj][:], ACTF.Sigmoid,
                                            bias=bias1,
                                        )
                                        nc.vector.scalar_tensor_tensor(
                                            act[:], ps1[jj][:], bias1, act[:],
                                            op0=ALU.add, op1=ALU.mult,
                                        )
                                    dst = (qt_sb if s == "q" else kt_sb)[
                                        :, j, t * 512:(t + 1) * 512
                                    ]
                                    nc.vector.scalar_tensor_tensor(
                                        dst, ps2[jj][:], bias2, act[:],
                                        op0=ALU.add, op1=ALU.mult,
                                    )

            # ------------- Phase B+C: attention + output projection -------
            with (
                tc.tile_pool(name="scps", bufs=2, space="PSUM") as scps,
                tc.tile_pool(name="cxps", bufs=2, space="PSUM") as cxps,
                tc.tile_pool(name="bcps", bufs=2, space="PSUM") as bcps,
                tc.tile_pool(name="apool", bufs=4) as apool,
                tc.tile_pool(name="ctpool", bufs=2) as ctpool,
                tc.tile_pool(name="smalls", bufs=4) as smalls,
                tc.tile_pool(name="ostage", bufs=4) as ostage,
                tc.tile_pool(name="mpool", bufs=2) as mpool,
            ):
                for qg in range(4):
                    kcmax = kc_count(qg)
                    qsl = slice(qg * 512, (qg + 1) * 512)
                    ct_qg = ctpool.tile([128, 4, 512], F32R, tag="ct")

                    mtiles = None
                    if mask_mode == "general":
                        mtiles = []
                        mt_sb = mpool.tile([128, NT, 512], F32, tag="mt")
                        for kc in range(kcmax):
                            nc.sync.dma_start(
                                mt_sb[:, kc, :],
                                m01T_d[kc * 128:(kc + 1) * 128, qsl],
                            )
                            mtiles.append(mt_sb[:, kc, :])

                    for pj in range(4):   # head pair: hl = 2*pj (+1)
                        ctx = [cxps.tile([128, 512], F32, tag="cx",
                                         name=f"ctx_{i}")
                               for i in range(2)]
                        for kk in range((kcmax + 1) // 2):
                            sc = [scps.tile([128, 1024], F32, tag="sc",
                                            name=f"sc_{i}")
                                  for i in range(2)]
                            for half in range(2):
                                kc = 2 * kk + half
                                if kc >= kcmax:
                                    continue
                                ksl = slice(kc * 128, (kc + 1) * 128)
                                for par in range(2):
                                    bp = par * 64
                                    nc.tensor.matmul(
                                        sc[par][:, half * 512:(half + 1) * 512],
                                        kt_sb[bp:bp + 64, pj, ksl],
                                        qt_sb[bp:bp + 64, pj, qsl],
                                    )
                            nhalf = min(2, kcmax - 2 * kk)
                            for par in range(2):
                                attn = apool.tile([128, 1024], F32R, tag="at")
                                nc.scalar.activation(
                                    attn[:, 0:nhalf * 512],
                                    sc[par][:, 0:nhalf * 512],
                                    ACTF.Exp,
                                )
                                for half in range(nhalf):
                                    kc = 2 * kk + half
                                    hsl = slice(half * 512, (half + 1) * 512)
                                    if mask_mode == "causal" and kc >= 4 * qg:
                                        nc.vector.tensor_tensor(
                                            attn[:, hsl], attn[:, hsl],
                                            pat_sb[:, kc - 4 * qg, :],
                                            ALU.mult,
                                        )
                                    elif mask_mode == "general":
                                        nc.vector.tensor_tensor(
                                            attn[:, hsl], attn[:, hsl],
                                            mtiles[kc], ALU.mult,
                                        )
                                for half in range(nhalf):
                                    kc = 2 * kk + half
                                    hl = 2 * pj + par
                                    nc.tensor.matmul(
                                        ctx[par][0:65, :],
                                        v_sb[:, kc, hl, :],
                                        attn[:, half * 512:(half + 1) * 512],
                                        start=(kc == 0),
                                        stop=(kc == kcmax - 1),
                                    )
                        # normalize both heads of the pair into ct_qg
                        for par in range(2):
                            den = smalls.tile([1, 512], F32, tag="den")
                            nc.vector.tensor_copy(den[:], ctx[par][64:65, :])
                            rec = smalls.tile([1, 512], F32R, tag="rec")
                            with nc.allow_low_precision(reason="f32r==fp32"):
                                nc.vector.reciprocal(rec[:], den[:])
                            bc_ps = bcps.tile([128, 512], F32, tag="bc")
                            nc.tensor.matmul(
                                bc_ps[0:64, :], ones_r[0:1, 0:64], rec[:]
                            )
                            bc_sb = smalls.tile([64, 512], F32, tag="bcs")
                            nc.vector.tensor_copy(bc_sb[:], bc_ps[0:64, :])
                            bp = par * 64
                            nc.vector.tensor_tensor(
                                ct_qg[bp:bp + 64, pj, :],
                                ctx[par][0:64, :], bc_sb[:], ALU.mult,
                            )

                    # ---- output projection for this q-group ----
                    for ns in range(4):
                        nt_i = qg * 4 + ns
                        nsl = slice(ns * 128, (ns + 1) * 128)
                        for oh in range(2):
                            po = bcps.tile([128, 512], F32, tag="bc")
                            for j in range(4):
                                nc.tensor.matmul(
                                    po[:],
                                    ct_qg[:, j, nsl],
                                    woT_sb[:, j, oh * 512:(oh + 1) * 512],
                                    start=(j == 0), stop=(j == 3),
                                )
                            ot = ostage.tile([128, 512], F32, tag="ot")
                            nc.vector.tensor_copy(ot[:], po[:])
                            nc.sync.dma_start(
                                pout_d[nt_i * 128:(nt_i + 1) * 128,
                                       oh * 512:(oh + 1) * 512],
                                ot[:],
                            )
    nc.compile()
    return nc


def _host_prepare(inputs):
    """Split the full problem into 8 per-core input maps + host-side info."""
    q = np.asarray(inputs["query"], dtype=np.float32)
    k = np.asarray(inputs["key"], dtype=np.float32)
    v = np.asarray(inputs["value"], dtype=np.float32)
    mask = np.asarray(inputs["mask"])
    w = {n: np.asarray(inputs[n], dtype=np.float32)
         for n in ("wq1", "wq2", "wk1", "wk2", "wv1", "wv2", "wo")}
    bias = {n: np.asarray(inputs[n], dtype=np.float32)
            for n in ("bq1", "bq2", "bk1", "bk2", "bv1", "bv2", "bo")}

    m = mask.reshape(S, S)
    if np.array_equal(m != 0, np.tril(np.ones((S, S), bool))):
        mask_mode = "causal"
    elif np.all(m != 0):
        mask_mode = "full"
    else:
        mask_mode = "general"

    pat = None
    m01T = None
    if mask_mode == "causal":
        kk = np.arange(128)[:, None]
        qq = np.arange(512)[None, :]
        pat = np.stack(
            [(kk + 128 * i <= qq).astype(np.float32) for i in range(4)], axis=1
        )  # [128, 4, 512]
        pat = np.ascontiguousarray(pat)
    elif mask_mode == "general":
        m01T = np.ascontiguousarray((m != 0).T.astype(np.float32))

    scale = 1.0 / np.sqrt(DK).astype(np.float32)

    in_maps = []
    for c in range(NCORES):
        b, g = divmod(c, 2)
        sl = slice(g * GCH, (g + 1) * GCH)
        im = {
            "xqT": np.ascontiguousarray(q[b].T),
            "xkT": np.ascontiguousarray(k[b].T),
            "xvT": np.ascontiguousarray(v[b].T),
            "w1T_q": np.ascontiguousarray(w["wq1"][sl].T),
            # fold the 1/sqrt(dk) score scale into the non-silu Q branch
            "w2T_q": np.ascontiguousarray(w["wq2"][sl].T) * scale,
            "w1T_k": np.ascontiguousarray(w["wk1"][sl].T),
            "w2T_k": np.ascontiguousarray(w["wk2"][sl].T),
            "w1T_v": np.ascontiguousarray(w["wv1"][sl].T),
            "w2T_v": np.ascontiguousarray(w["wv2"][sl].T),
            "b1_q": np.ascontiguousarray(bias["bq1"][sl].reshape(4, 128).T),
            "b2_q": np.ascontiguousarray(
                (bias["bq2"][sl] * scale).reshape(4, 128).T),
            "b1_k": np.ascontiguousarray(bias["bk1"][sl].reshape(4, 128).T),
            "b2_k": np.ascontiguousarray(bias["bk2"][sl].reshape(4, 128).T),
            "b1_v": np.ascontiguousarray(bias["bv1"][sl].reshape(1, GCH)),
            "b2_v": np.ascontiguousarray(bias["bv2"][sl].reshape(1, GCH)),
            "woT": np.ascontiguousarray(
                w["wo"][:, sl].T.reshape(4, 128, D).transpose(1, 0, 2)),
        }
        if mask_mode == "causal":
            im["pat"] = pat
        elif mask_mode == "general":
            im["m01T"] = m01T
        in_maps.append(im)
    return mask_mode, in_maps, bias["bo"]


def kernel(**inputs):
    global LAST_RESULT
    mask_mode, in_maps, bo = _host_prepare(inputs)
    nc = build_program(mask_mode)

    import concourse.bass_utils as bu

    if TRACE:
        import types

        try:
            from trn_agent_boot.trn_boot import _ntff_profile_via_ctypes

            hook = _ntff_profile_via_ctypes("/opt/axon/libaxon_pjrt.so")
            m = types.ModuleType("antenv.axon_hooks")
            m.get_axon_ntff_profile_hook = lambda: hook
            import antenv  # noqa: F401

            sys.modules["antenv.axon_hooks"] = m
            bu.upload_artifacts = lambda d: "local://skipped"
        except Exception as e:
            print("profiling hook install failed:", e)

    res = bu.run_bass_kernel_spmd(
        nc, in_maps, core_ids=list(range(NCORES)),
        trace=TRACE, trace_cores=TRACE_CORES,
    )
    LAST_RESULT = res

    out = np.empty((B, S, D), dtype=np.float32)
    for b in range(B):
        out[b] = (res.results[2 * b]["pout"] + res.results[2 * b + 1]["pout"]
                  + bo[None, :])
    return out


# revision 8
# speedup vs baseline: 1.0017x; 1.0017x over previous
"""SwiGLU-projected causal MHA (B=4, S=2048, D=1024, H=16) on 8 TRN2 NeuronCores.

Sharding: core c -> (batch b = c//2, head-group g = c%2).  Each core computes
the SwiGLU Q/K/V projections for its 512 output channels (= 8 heads) of its
batch, runs causal attention for those heads, and produces a partial output
projection (contraction over its 512 channels).  The host sums the two
partials per batch and adds the output bias.

Device layout (per core):
  QT/KT [128p, 4j, 2048n]   channels on partitions (local ch = j*128 + p),
                            seq on free.  Head hl -> chunk hl//2, partition
                            base 64*(hl%2); head pairs run concurrently on
                            the PE via disjoint row groups (K=64 matmuls at
                            base 0 / base 64).
  V     [128p, 16nt, 8hl, 65]  seq on partitions (n = nt*128+p); per head 64
                            channels plus a ones column, so the AV matmul
                            emits the softmax denominator as output row 64.
  Scores are computed transposed, S^T [k-part, q-free], exp'd on the scalar
  engine straight out of PSUM (no max subtraction -- logits are bounded),
  masked multiplicatively on diagonal blocks only, and consumed as the
  moving operand of the AV matmul.  All matmuls run in float32r (full-rate
  fp32 streaming; measured ~1.6e-4 relative error on HW).
"""
import sys

sys.path.insert(0, "/opt/trn_rl_repo")
import numpy as np

import concourse.bacc as bacc
import concourse.tile as tile
import concourse.mybir as mybir

B, S, D = 4, 2048, 1024
H, DK = 16, 64
NCORES = 8
GCH = 512          # channels per core (8 heads)
NT = S // 128      # 16 seq chunks
F32 = mybir.dt.float32
F32R = mybir.dt.float32r
ACTF = mybir.ActivationFunctionType
ALU = mybir.AluOpType

TRACE = False          # set by test.py for profiling runs
TRACE_CORES = None
LAST_RESULT = None     # BassKernelResults stash for test.py
USE_SILU = True        # HW has a Silu table; CoreSim only has Sigmoid


def build_program(mask_mode):
    """mask_mode: 'causal' (tril), 'full' (all ones), 'general' (arbitrary)."""
    nc = bacc.Bacc("TRN2", target_bir_lowering=False, debug=False)

    xT = {s: nc.dram_tensor(f"x{s}T", [D, S], F32R, kind="ExternalInput")
          for s in "qkv"}
    w1T = {s: nc.dram_tensor(f"w1T_{s}", [D, GCH], F32R, kind="ExternalInput")
           for s in "qkv"}
    w2T = {s: nc.dram_tensor(f"w2T_{s}", [D, GCH], F32R, kind="ExternalInput")
           for s in "qkv"}
    bias_d = {}
    for s in "qk":
        for bn in ("b1", "b2", "b1h"):
            bias_d[f"{bn}_{s}"] = nc.dram_tensor(f"{bn}_{s}", [128, 4], F32,
                                                 kind="ExternalInput")
    b1v_d = nc.dram_tensor("b1_v", [1, GCH], F32R, kind="ExternalInput")
    b2v_d = nc.dram_tensor("b2_v", [1, GCH], F32R, kind="ExternalInput")
    woT_d = nc.dram_tensor("woT", [128, 4, D], F32R, kind="ExternalInput")
    pat_d = m01T_d = None
    if mask_mode == "causal":
        pat_d = nc.dram_tensor("pat", [128, 4, 512], F32, kind="ExternalInput")
    elif mask_mode == "general":
        m01T_d = nc.dram_tensor("m01T", [S, S], F32, kind="ExternalInput")
    pout_d = nc.dram_tensor("pout", [S, D], F32, kind="ExternalOutput")

    def kc_count(qg):
        return 4 * qg + 4 if mask_mode == "causal" else NT

    with tile.TileContext(nc) as tc:
        with (
            tc.tile_pool(name="persist", bufs=1) as persist,
        ):
            qt_sb = persist.tile([128, 4, S], F32R, tag="qt")
            kt_sb = persist.tile([128, 4, S], F32R, tag="kt")
            v_sb = persist.tile([128, NT, 8, 65], F32R, tag="v")
            woT_sb = persist.tile([128, 4, D], F32R, tag="wo")
            nc.sync.dma_start(woT_sb[:], woT_d[:])
            onesf = persist.tile([1, 128], F32, tag="onesf")
            ones_r = persist.tile([1, 128], F32R, tag="ones_r")
            nc.any.memset(onesf[:], 1.0)
            nc.vector.tensor_copy(ones_r[:], onesf[:])
            onescol = persist.tile([128, 1], F32, tag="onescol")
            nc.any.memset(onescol[:], 1.0)
            nc.vector.tensor_copy(
                v_sb[:, :, :, 64:65],
                onescol[:, None, :].to_broadcast([128, NT, 8, 1]),
            )
            if mask_mode == "causal":
                pat_sb = persist.tile([128, 4, 512], F32, tag="pat")
                nc.sync.dma_start(pat_sb[:], pat_d[:])

            # ---------------- Phase A: SwiGLU projections ----------------
            with (
                tc.tile_pool(name="wpool", bufs=2) as wpool,
                tc.tile_pool(name="xpool", bufs=9) as xpool,
                tc.tile_pool(name="stage", bufs=4) as stage,
                tc.tile_pool(name="pps", bufs=6, space="PSUM") as pps,
            ):
                for s in "qkv":
                    w1sb = wpool.tile([128, 8, GCH], F32R, tag="w")
                    w2sb = wpool.tile([128, 8, GCH], F32R, tag="w")
                    nc.sync.dma_start(
                        w1sb[:], w1T[s][:].rearrange("(dc p) o -> p dc o", p=128)
                    )
                    nc.sync.dma_start(
                        w2sb[:], w2T[s][:].rearrange("(dc p) o -> p dc o", p=128)
                    )
                    if s != "v":
                        b1sb = persist.tile([128, 4], F32, tag=f"b1{s}")
                        b2sb = persist.tile([128, 4], F32, tag=f"b2{s}")
                        b1hsb = persist.tile([128, 4], F32, tag=f"b1h{s}")
                        nc.sync.dma_start(b1sb[:], bias_d[f"b1_{s}"][:])
                        nc.sync.dma_start(b2sb[:], bias_d[f"b2_{s}"][:])
                        nc.sync.dma_start(b1hsb[:], bias_d[f"b1h_{s}"][:])
                    else:
                        b1vr = persist.tile([1, GCH], F32R, tag="b1v")
                        b2vr = persist.tile([1, GCH], F32R, tag="b2v")
                        nc.sync.dma_start(b1vr[:], b1v_d[:])
                        nc.sync.dma_start(b2vr[:], b2v_d[:])

                    for t in range(4):  # 512-wide seq tiles
                        xts = []
                        for dc in range(8):
                            xt = xpool.tile([128, 512], F32R, tag="xt")
                            nc.sync.dma_start(
                                xt[:],
                                xT[s][dc * 128:(dc + 1) * 128,
                                      t * 512:(t + 1) * 512],
                            )
                            xts.append(xt)
                        for jh in range(2):
                            ps1 = [pps.tile([128, 512], F32, tag="pp",
                                            name=f"ps1_{i}")
                                   for i in range(2)]
                            ps2 = [pps.tile([128, 512], F32, tag="pp",
                                            name=f"ps2_{i}")
                                   for i in range(2)]
                            for dc in range(8):
                                for jj in range(2):
                                    j = jh * 2 + jj
                                    if s == "v":
                                        # seq on partitions: lhsT = x chunk
                                        nc.tensor.matmul(
                                            ps1[jj][:],
                                            xts[dc][:, j * 128:(j + 1) * 128],
                                            w1sb[:, dc, :],
                                            start=(dc == 0), stop=False,
                                        )
                                        nc.tensor.matmul(
                                            ps2[jj][:],
                                            xts[dc][:, j * 128:(j + 1) * 128],
                                            w2sb[:, dc, :],
                                            start=(dc == 0), stop=False,
                                        )
                                    else:
                                        # channels on partitions: lhsT = w chunk
                                        nc.tensor.matmul(
                                            ps1[jj][:],
                                            w1sb[:, dc, j * 128:(j + 1) * 128],
                                            xts[dc][:],
                                            start=(dc == 0), stop=(dc == 7),
                                        )
                                        nc.tensor.matmul(
                                            ps2[jj][:],
                                            w2sb[:, dc, j * 128:(j + 1) * 128],
                                            xts[dc][:],
                                            start=(dc == 0), stop=(dc == 7),
                                        )
                            for jj in range(2):
                                j = jh * 2 + jj
                                act = stage.tile([128, 512], F32, tag="act")
                                if s == "v":
                                    # fold the biases into the accumulation
                                    # (they vary along the free/channel dim)
                                    nc.tensor.matmul(
                                        ps1[jj][:], ones_r[:], b1vr[:],
                                        start=False, stop=True,
                                    )
                                    nc.tensor.matmul(
                                        ps2[jj][:], ones_r[:], b2vr[:],
                                        start=False, stop=True,
                                    )
                                    nc.scalar.activation(
                                        act[:], ps1[jj][:], ACTF.Tanh,
                                        scale=0.5,
                                    )
                                    u = stage.tile([128, 512], F32, tag="u")
                                    nc.vector.tensor_tensor(
                                        u[:], ps1[jj][:], act[:], ALU.mult
                                    )
                                    nc.vector.tensor_tensor(
                                        act[:], ps1[jj][:], u[:], ALU.add
                                    )
                                    nt_i = t * 4 + j
                                    nc.vector.tensor_tensor(
                                        v_sb[:, nt_i, :, 0:64],
                                        ps2[jj][:].rearrange(
                                            "p (h d) -> p h d", h=8
                                        ),
                                        act[:].rearrange(
                                            "p (h d) -> p h d", h=8
                                        ),
                                        ALU.mult,
                                    )
                                else:
                                    bias1 = b1sb[:, j:j + 1]
                                    bias2 = b2sb[:, j:j + 1]
                                    # act = tanh((A)/2), A = ps1 + b1
                                    nc.scalar.activation(
                                        act[:], ps1[jj][:], ACTF.Tanh,
                                        scale=0.5, bias=b1hsb[:, j:j + 1],
                                    )
                                    a_sb = stage.tile([128, 512], F32,
                                                      tag="u")
                                    nc.vector.tensor_scalar_add(
                                        a_sb[:], ps1[jj][:], bias1
                                    )
                                    # act = A*(1+tanh(A/2)) = 2*silu(A)
                                    nc.vector.scalar_tensor_tensor(
                                        act[:], act[:], 1.0, a_sb[:],
                                        op0=ALU.add, op1=ALU.mult,
                                    )
                                    dst = (qt_sb if s == "q" else kt_sb)[
                                        :, j, t * 512:(t + 1) * 512
                                    ]
                                    nc.vector.scalar_tensor_tensor(
                                        dst, ps2[jj][:], bias2, act[:],
                                        op0=ALU.add, op1=ALU.mult,
                                    )

            # ------------- Phase B+C: attention + output projection -------
            # Order all projection work (ACT: Silu) before attention work
            # (ACT: Exp) so the activation table set switches only once.
            tc.no_sync_barrier()
            with (
                tc.tile_pool(name="scps", bufs=2, space="PSUM") as scps,
                tc.tile_pool(name="cxps", bufs=2, space="PSUM") as cxps,
                tc.tile_pool(name="bcps", bufs=2, space="PSUM") as bcps,
                tc.tile_pool(name="apool", bufs=4) as apool,
                tc.tile_pool(name="ctpool", bufs=2) as ctpool,
                tc.tile_pool(name="smalls", bufs=4) as smalls,
                tc.tile_pool(name="ostage", bufs=4) as ostage,
                tc.tile_pool(name="mpool", bufs=2) as mpool,
            ):
                for qg in range(4):
                    kcmax = kc_count(qg)
                    qsl = slice(qg * 512, (qg + 1) * 512)
                    ct_qg = ctpool.tile([128, 4, 512], F32R, tag="ct")

                    mtiles = None
                    if mask_mode == "general":
                        mtiles = []
                        mt_sb = mpool.tile([128, NT, 512], F32, tag="mt")
                        for kc in range(kcmax):
                            nc.sync.dma_start(
                                mt_sb[:, kc, :],
                                m01T_d[kc * 128:(kc + 1) * 128, qsl],
                            )
                            mtiles.append(mt_sb[:, kc, :])

                    for pj in range(4):   # head pair: hl = 2*pj (+1)
                        ctx = [cxps.tile([128, 512], F32, tag="cx",
                                         name=f"ctx_{i}")
                               for i in range(2)]
                        for kk in range((kcmax + 1) // 2):
                            sc = [scps.tile([128, 1024], F32, tag="sc",
                                            name=f"sc_{i}")
                                  for i in range(2)]
                            for half in range(2):
                                kc = 2 * kk + half
                                if kc >= kcmax:
                                    continue
                                ksl = slice(kc * 128, (kc + 1) * 128)
                                for par in range(2):
                                    bp = par * 64
                                    nc.tensor.matmul(
                                        sc[par][:, half * 512:(half + 1) * 512],
                                        kt_sb[bp:bp + 64, pj, ksl],
                                        qt_sb[bp:bp + 64, pj, qsl],
                                    )
                            nhalf = min(2, kcmax - 2 * kk)
                            for par in range(2):
                                attn = apool.tile([128, 1024], F32R, tag="at")
                                nc.scalar.activation(
                                    attn[:, 0:nhalf * 512],
                                    sc[par][:, 0:nhalf * 512],
                                    ACTF.Exp,
                                )
                                for half in range(nhalf):
                                    kc = 2 * kk + half
                                    hsl = slice(half * 512, (half + 1) * 512)
                                    if mask_mode == "causal" and kc >= 4 * qg:
                                        nc.gpsimd.tensor_tensor(
                                            attn[:, hsl], attn[:, hsl],
                                            pat_sb[:, kc - 4 * qg, :],
                                            ALU.mult,
                                        )
                                    elif mask_mode == "general":
                                        nc.gpsimd.tensor_tensor(
                                            attn[:, hsl], attn[:, hsl],
                                            mtiles[kc], ALU.mult,
                                        )
                                for half in range(nhalf):
                                    kc = 2 * kk + half
                                    hl = 2 * pj + par
                                    nc.tensor.matmul(
                                        ctx[par][0:65, :],
                                        v_sb[:, kc, hl, :],
                                        attn[:, half * 512:(half + 1) * 512],
                                        start=(kc == 0),
                                        stop=(kc == kcmax - 1),
                                    )
                        # normalize both heads of the pair into ct_qg.
                        # One batched reciprocal; denominator rows live at
                        # partitions 0 and 32 (the only legal operand bases).
                        den = smalls.tile([33, 512], F32, tag="den")
                        nc.gpsimd.memset(den[:], 1.0)
                        for par in range(2):
                            nc.vector.tensor_copy(
                                den[32 * par:32 * par + 1, :],
                                ctx[par][64:65, :],
                            )
                        rec = smalls.tile([33, 512], F32R, tag="rec")
                        with nc.allow_low_precision(reason="f32r==fp32"):
                            nc.vector.reciprocal(rec[:], den[:])
                        recb = smalls.tile([1, 512], F32R, tag="recb")
                        nc.vector.tensor_copy(recb[:], rec[32:33, :])
                        rec_rows = (rec[0:1, :], recb[:])
                        for par in range(2):
                            bc_ps = bcps.tile([128, 512], F32, tag="bc")
                            nc.tensor.matmul(
                                bc_ps[0:64, :], ones_r[0:1, 0:64],
                                rec_rows[par],
                            )
                            bc_sb = smalls.tile([64, 512], F32, tag="bcs")
                            nc.vector.tensor_copy(bc_sb[:], bc_ps[0:64, :])
                            bp = par * 64
                            nc.vector.tensor_tensor(
                                ct_qg[bp:bp + 64, pj, :],
                                ctx[par][0:64, :], bc_sb[:], ALU.mult,
                            )

                    # ---- output projection for this q-group ----
                    for ns in range(4):
                        nt_i = qg * 4 + ns
                        nsl = slice(ns * 128, (ns + 1) * 128)
                        for oh in range(2):
                            po = bcps.tile([128, 512], F32, tag="bc")
                            for j in range(4):
                                nc.tensor.matmul(
                                    po[:],
                                    ct_qg[:, j, nsl],
                                    woT_sb[:, j, oh * 512:(oh + 1) * 512],
                                    start=(j == 0), stop=(j == 3),
                                )
                            ot = ostage.tile([128, 512], F32, tag="ot")
                            nc.vector.tensor_copy(ot[:], po[:])
                            nc.sync.dma_start(
                                pout_d[nt_i * 128:(nt_i + 1) * 128,
                                       oh * 512:(oh + 1) * 512],
                                ot[:],
                            )
    nc.compile()
    return nc


def _host_prepare(inputs):
    """Split the full problem into 8 per-core input maps + host-side info."""
    q = np.asarray(inputs["query"], dtype=np.float32)
    k = np.asarray(inputs["key"], dtype=np.float32)
    v = np.asarray(inputs["value"], dtype=np.float32)
    mask = np.asarray(inputs["mask"])
    w = {n: np.asarray(inputs[n], dtype=np.float32)
         for n in ("wq1", "wq2", "wk1", "wk2", "wv1", "wv2", "wo")}
    bias = {n: np.asarray(inputs[n], dtype=np.float32)
            for n in ("bq1", "bq2", "bk1", "bk2", "bv1", "bv2", "bo")}

    m = mask.reshape(S, S)
    if np.array_equal(m != 0, np.tril(np.ones((S, S), bool))):
        mask_mode = "causal"
    elif np.all(m != 0):
        mask_mode = "full"
    else:
        mask_mode = "general"

    pat = None
    m01T = None
    if mask_mode == "causal":
        kk = np.arange(128)[:, None]
        qq = np.arange(512)[None, :]
        pat = np.stack(
            [(kk + 128 * i <= qq).astype(np.float32) for i in range(4)], axis=1
        )  # [128, 4, 512]
        pat = np.ascontiguousarray(pat)
    elif mask_mode == "general":
        m01T = np.ascontiguousarray((m != 0).T.astype(np.float32))

    scale = 1.0 / np.sqrt(DK).astype(np.float32)

    in_maps = []
    for c in range(NCORES):
        b, g = divmod(c, 2)
        sl = slice(g * GCH, (g + 1) * GCH)
        im = {
            "xqT": np.ascontiguousarray(q[b].T),
            "xkT": np.ascontiguousarray(k[b].T),
            "xvT": np.ascontiguousarray(v[b].T),
            "w1T_q": np.ascontiguousarray(w["wq1"][sl].T),
            # fold the 1/sqrt(dk) score scale into the non-silu Q branch,
            # and 0.5 everywhere (silu computed as A*(1+tanh(A/2)) = 2*silu)
            "w2T_q": np.ascontiguousarray(w["wq2"][sl].T) * (scale * 0.5),
            "w2T_k": np.ascontiguousarray(w["wk2"][sl].T) * 0.5,
            "w2T_v": np.ascontiguousarray(w["wv2"][sl].T) * 0.5,
            "w1T_k": np.ascontiguousarray(w["wk1"][sl].T),
            "w1T_v": np.ascontiguousarray(w["wv1"][sl].T),
            "b1_q": np.ascontiguousarray(bias["bq1"][sl].reshape(4, 128).T),
            "b1h_q": np.ascontiguousarray(
                (bias["bq1"][sl] * 0.5).reshape(4, 128).T),
            "b2_q": np.ascontiguousarray(
                (bias["bq2"][sl] * (scale * 0.5)).reshape(4, 128).T),
            "b1_k": np.ascontiguousarray(bias["bk1"][sl].reshape(4, 128).T),
            "b1h_k": np.ascontiguousarray(
                (bias["bk1"][sl] * 0.5).reshape(4, 128).T),
            "b2_k": np.ascontiguousarray(
                (bias["bk2"][sl] * 0.5).reshape(4, 128).T),
            "b1_v": np.ascontiguousarray(bias["bv1"][sl].reshape(1, GCH)),
            "b2_v": np.ascontiguousarray(
                (bias["bv2"][sl] * 0.5).reshape(1, GCH)),
            "woT": np.ascontiguousarray(
                w["wo"][:, sl].T.reshape(4, 128, D).transpose(1, 0, 2)),
        }
        if mask_mode == "causal":
            im["pat"] = pat
        elif mask_mode == "general":
            im["m01T"] = m01T
        in_maps.append(im)
    return mask_mode, in_maps, bias["bo"]


def kernel(**inputs):
    global LAST_RESULT
    mask_mode, in_maps, bo = _host_prepare(inputs)
    nc = build_program(mask_mode)

    import concourse.bass_utils as bu

    if TRACE:
        import types

        try:
            from trn_agent_boot.trn_boot import _ntff_profile_via_ctypes

            hook = _ntff_profile_via_ctypes("/opt/axon/libaxon_pjrt.so")
            m = types.ModuleType("antenv.axon_hooks")
            m.get_axon_ntff_profile_hook = lambda: hook
            import antenv  # noqa: F401

            sys.modules["antenv.axon_hooks"] = m
            bu.upload_artifacts = lambda d: "local://skipped"
        except Exception as e:
            print("profiling hook install failed:", e)

    res = bu.run_bass_kernel_spmd(
        nc, in_maps, core_ids=list(range(NCORES)),
        trace=TRACE, trace_cores=TRACE_CORES,
    )
    LAST_RESULT = res

    out = np.empty((B, S, D), dtype=np.float32)
    for b in range(B):
        out[b] = (res.results[2 * b]["pout"] + res.results[2 * b + 1]["pout"]
                  + bo[None, :])
    return out


# revision 9
# speedup vs baseline: 1.1248x; 1.1229x over previous
"""SwiGLU-projected causal MHA (B=4, S=2048, D=1024, H=16) on 8 TRN2 NeuronCores.

Sharding: core c -> (batch b = c//2, head-group g = c%2).  Each core computes
the SwiGLU Q/K/V projections for its 512 output channels (= 8 heads) of its
batch, runs causal attention for those heads, and produces a partial output
projection (contraction over its 512 channels).  The host sums the two
partials per batch and adds the output bias.

Device layout (per core):
  QT/KT [128p, 4j, 2048n]   channels on partitions (local ch = j*128 + p),
                            seq on free.  Head hl -> chunk hl//2, partition
                            base 64*(hl%2); head pairs run concurrently on
                            the PE via disjoint row groups (K=64 matmuls at
                            base 0 / base 64).
  V     [128p, 16nt, 8hl, 65]  seq on partitions (n = nt*128+p); per head 64
                            channels plus a ones column, so the AV matmul
                            emits the softmax denominator as output row 64.
  Scores are computed transposed, S^T [k-part, q-free], exp'd on the scalar
  engine straight out of PSUM (no max subtraction -- logits are bounded),
  masked multiplicatively on diagonal blocks only, and consumed as the
  moving operand of the AV matmul.  All matmuls run in float32r (full-rate
  fp32 streaming; measured ~1.6e-4 relative error on HW).
"""
import sys

sys.path.insert(0, "/opt/trn_rl_repo")
import numpy as np

import concourse.bacc as bacc
import concourse.tile as tile
import concourse.mybir as mybir

B, S, D = 4, 2048, 1024
H, DK = 16, 64
NCORES = 8
GCH = 512          # channels per core (8 heads)
NT = S // 128      # 16 seq chunks
F32 = mybir.dt.float32
F32R = mybir.dt.float32r
ACTF = mybir.ActivationFunctionType
ALU = mybir.AluOpType

TRACE = False          # set by test.py for profiling runs
TRACE_CORES = None
LAST_RESULT = None     # BassKernelResults stash for test.py
MM_DTYPE = "bf16"      # "bf16" (fast weight load) or "f32r" (higher precision)


def build_program(mask_mode):
    """mask_mode: 'causal' (tril), 'full' (all ones), 'general' (arbitrary)."""
    MMD = mybir.dt.bfloat16 if MM_DTYPE == "bf16" else F32R
    nc = bacc.Bacc("TRN2", target_bir_lowering=False, debug=False)

    xT = {s: nc.dram_tensor(f"x{s}T", [D, S], MMD, kind="ExternalInput")
          for s in "qkv"}
    w1T = {s: nc.dram_tensor(f"w1T_{s}", [D, GCH], MMD, kind="ExternalInput")
           for s in "qkv"}
    w2T = {s: nc.dram_tensor(f"w2T_{s}", [D, GCH], MMD, kind="ExternalInput")
           for s in "qkv"}
    bias_d = {}
    for s in "qk":
        for bn in ("b1", "b2", "b1h"):
            bias_d[f"{bn}_{s}"] = nc.dram_tensor(f"{bn}_{s}", [128, 4], F32,
                                                 kind="ExternalInput")
    b1v_d = nc.dram_tensor("b1_v", [1, GCH], MMD, kind="ExternalInput")
    b2v_d = nc.dram_tensor("b2_v", [1, GCH], MMD, kind="ExternalInput")
    woT_d = nc.dram_tensor("woT", [128, 4, D], MMD, kind="ExternalInput")
    pat_d = m01T_d = None
    if mask_mode == "causal":
        pat_d = nc.dram_tensor("pat", [128, 4, 512], MMD, kind="ExternalInput")
    elif mask_mode == "general":
        m01T_d = nc.dram_tensor("m01T", [S, S], MMD, kind="ExternalInput")
    pout_d = nc.dram_tensor("pout", [S, D], F32, kind="ExternalOutput")

    def kc_count(qg):
        return 4 * qg + 4 if mask_mode == "causal" else NT

    with tile.TileContext(nc) as tc:
        with (
            tc.tile_pool(name="persist", bufs=1) as persist,
        ):
            qt_sb = persist.tile([128, 4, S], MMD, tag="qt")
            kt_sb = persist.tile([128, 4, S], MMD, tag="kt")
            v_sb = persist.tile([128, NT, 8, 65], MMD, tag="v")
            woT_sb = persist.tile([128, 4, D], MMD, tag="wo")
            nc.sync.dma_start(woT_sb[:], woT_d[:])
            onesf = persist.tile([1, 128], F32, tag="onesf")
            ones_r = persist.tile([1, 128], MMD, tag="ones_r")
            nc.any.memset(onesf[:], 1.0)
            nc.vector.tensor_copy(ones_r[:], onesf[:])
            onescol = persist.tile([128, 1], F32, tag="onescol")
            nc.any.memset(onescol[:], 1.0)
            nc.vector.tensor_copy(
                v_sb[:, :, :, 64:65],
                onescol[:, None, :].to_broadcast([128, NT, 8, 1]),
            )
            if mask_mode == "causal":
                pat_sb = persist.tile([128, 4, 512], MMD, tag="pat")
                nc.sync.dma_start(pat_sb[:], pat_d[:])

            # ---------------- Phase A: SwiGLU projections ----------------
            with (
                tc.tile_pool(name="wpool", bufs=2) as wpool,
                tc.tile_pool(name="xpool", bufs=9) as xpool,
                tc.tile_pool(name="stage", bufs=4) as stage,
                tc.tile_pool(name="pps", bufs=6, space="PSUM") as pps,
            ):
                for s in "qkv":
                    w1sb = wpool.tile([128, 8, GCH], MMD, tag="w")
                    w2sb = wpool.tile([128, 8, GCH], MMD, tag="w")
                    nc.sync.dma_start(
                        w1sb[:], w1T[s][:].rearrange("(dc p) o -> p dc o", p=128)
                    )
                    nc.sync.dma_start(
                        w2sb[:], w2T[s][:].rearrange("(dc p) o -> p dc o", p=128)
                    )
                    if s != "v":
                        b1sb = persist.tile([128, 4], F32, tag=f"b1{s}")
                        b2sb = persist.tile([128, 4], F32, tag=f"b2{s}")
                        b1hsb = persist.tile([128, 4], F32, tag=f"b1h{s}")
                        nc.sync.dma_start(b1sb[:], bias_d[f"b1_{s}"][:])
                        nc.sync.dma_start(b2sb[:], bias_d[f"b2_{s}"][:])
                        nc.sync.dma_start(b1hsb[:], bias_d[f"b1h_{s}"][:])
                    else:
                        b1vr = persist.tile([1, GCH], MMD, tag="b1v")
                        b2vr = persist.tile([1, GCH], MMD, tag="b2v")
                        nc.sync.dma_start(b1vr[:], b1v_d[:])
                        nc.sync.dma_start(b2vr[:], b2v_d[:])

                    for t in range(4):  # 512-wide seq tiles
                        xts = []
                        for dc in range(8):
                            xt = xpool.tile([128, 512], MMD, tag="xt")
                            nc.sync.dma_start(
                                xt[:],
                                xT[s][dc * 128:(dc + 1) * 128,
                                      t * 512:(t + 1) * 512],
                            )
                            xts.append(xt)
                        for jh in range(2):
                            ps1 = [pps.tile([128, 512], F32, tag="pp",
                                            name=f"ps1_{i}")
                                   for i in range(2)]
                            ps2 = [pps.tile([128, 512], F32, tag="pp",
                                            name=f"ps2_{i}")
                                   for i in range(2)]
                            for dc in range(8):
                                for jj in range(2):
                                    j = jh * 2 + jj
                                    if s == "v":
                                        # seq on partitions: lhsT = x chunk
                                        nc.tensor.matmul(
                                            ps1[jj][:],
                                            xts[dc][:, j * 128:(j + 1) * 128],
                                            w1sb[:, dc, :],
                                            start=(dc == 0), stop=False,
                                        )
                                        nc.tensor.matmul(
                                            ps2[jj][:],
                                            xts[dc][:, j * 128:(j + 1) * 128],
                                            w2sb[:, dc, :],
                                            start=(dc == 0), stop=False,
                                        )
                                    else:
                                        # channels on partitions: lhsT = w chunk
                                        nc.tensor.matmul(
                                            ps1[jj][:],
                                            w1sb[:, dc, j * 128:(j + 1) * 128],
                                            xts[dc][:],
                                            start=(dc == 0), stop=(dc == 7),
                                        )
                                        nc.tensor.matmul(
                                            ps2[jj][:],
                                            w2sb[:, dc, j * 128:(j + 1) * 128],
                                            xts[dc][:],
                                            start=(dc == 0), stop=(dc == 7),
                                        )
                            for jj in range(2):
                                j = jh * 2 + jj
                                act = stage.tile([128, 512], F32, tag="act")
                                if s == "v":
                                    # fold the biases into the accumulation
                                    # (they vary along the free/channel dim)
                                    nc.tensor.matmul(
                                        ps1[jj][:], ones_r[:], b1vr[:],
                                        start=False, stop=True,
                                    )
                                    nc.tensor.matmul(
                                        ps2[jj][:], ones_r[:], b2vr[:],
                                        start=False, stop=True,
                                    )
                                    nc.scalar.activation(
                                        act[:], ps1[jj][:], ACTF.Tanh,
                                        scale=0.5,
                                    )
                                    u = stage.tile([128, 512], F32, tag="u")
                                    nc.vector.tensor_tensor(
                                        u[:], ps1[jj][:], act[:], ALU.mult
                                    )
                                    nc.vector.tensor_tensor(
                                        act[:], ps1[jj][:], u[:], ALU.add
                                    )
                                    nt_i = t * 4 + j
                                    nc.vector.tensor_tensor(
                                        v_sb[:, nt_i, :, 0:64],
                                        ps2[jj][:].rearrange(
                                            "p (h d) -> p h d", h=8
                                        ),
                                        act[:].rearrange(
                                            "p (h d) -> p h d", h=8
                                        ),
                                        ALU.mult,
                                    )
                                else:
                                    bias1 = b1sb[:, j:j + 1]
                                    bias2 = b2sb[:, j:j + 1]
                                    # act = tanh((A)/2), A = ps1 + b1
                                    nc.scalar.activation(
                                        act[:], ps1[jj][:], ACTF.Tanh,
                                        scale=0.5, bias=b1hsb[:, j:j + 1],
                                    )
                                    a_sb = stage.tile([128, 512], F32,
                                                      tag="u")
                                    nc.vector.tensor_scalar_add(
                                        a_sb[:], ps1[jj][:], bias1
                                    )
                                    # act = A*(1+tanh(A/2)) = 2*silu(A)
                                    nc.vector.scalar_tensor_tensor(
                                        act[:], act[:], 1.0, a_sb[:],
                                        op0=ALU.add, op1=ALU.mult,
                                    )
                                    dst = (qt_sb if s == "q" else kt_sb)[
                                        :, j, t * 512:(t + 1) * 512
                                    ]
                                    nc.vector.scalar_tensor_tensor(
                                        dst, ps2[jj][:], bias2, act[:],
                                        op0=ALU.add, op1=ALU.mult,
                                    )

            # ------------- Phase B+C: attention + output projection -------
            # Order all projection work (ACT: Silu) before attention work
            # (ACT: Exp) so the activation table set switches only once.
            tc.no_sync_barrier()
            with (
                tc.tile_pool(name="scps", bufs=2, space="PSUM") as scps,
                tc.tile_pool(name="cxps", bufs=2, space="PSUM") as cxps,
                tc.tile_pool(name="bcps", bufs=2, space="PSUM") as bcps,
                tc.tile_pool(name="apool", bufs=4) as apool,
                tc.tile_pool(name="ctpool", bufs=2) as ctpool,
                tc.tile_pool(name="smalls", bufs=4) as smalls,
                tc.tile_pool(name="ostage", bufs=4) as ostage,
                tc.tile_pool(name="mpool", bufs=2) as mpool,
            ):
                for qg in range(4):
                    kcmax = kc_count(qg)
                    qsl = slice(qg * 512, (qg + 1) * 512)
                    ct_qg = ctpool.tile([128, 4, 512], MMD, tag="ct")

                    mtiles = None
                    if mask_mode == "general":
                        mtiles = []
                        mt_sb = mpool.tile([128, NT, 512], MMD, tag="mt")
                        for kc in range(kcmax):
                            nc.sync.dma_start(
                                mt_sb[:, kc, :],
                                m01T_d[kc * 128:(kc + 1) * 128, qsl],
                            )
                            mtiles.append(mt_sb[:, kc, :])

                    for pj in range(4):   # head pair: hl = 2*pj (+1)
                        ctx = [cxps.tile([128, 512], F32, tag="cx",
                                         name=f"ctx_{i}")
                               for i in range(2)]
                        for kk in range((kcmax + 1) // 2):
                            sc = [scps.tile([128, 1024], F32, tag="sc",
                                            name=f"sc_{i}")
                                  for i in range(2)]
                            for half in range(2):
                                kc = 2 * kk + half
                                if kc >= kcmax:
                                    continue
                                ksl = slice(kc * 128, (kc + 1) * 128)
                                for par in range(2):
                                    bp = par * 64
                                    nc.tensor.matmul(
                                        sc[par][:, half * 512:(half + 1) * 512],
                                        kt_sb[bp:bp + 64, pj, ksl],
                                        qt_sb[bp:bp + 64, pj, qsl],
                                    )
                            nhalf = min(2, kcmax - 2 * kk)
                            for par in range(2):
                                attn = apool.tile([128, 1024], MMD, tag="at")
                                nc.scalar.activation(
                                    attn[:, 0:nhalf * 512],
                                    sc[par][:, 0:nhalf * 512],
                                    ACTF.Exp,
                                )
                                for half in range(nhalf):
                                    kc = 2 * kk + half
                                    hsl = slice(half * 512, (half + 1) * 512)
                                    if mask_mode == "causal" and kc >= 4 * qg:
                                        nc.gpsimd.tensor_tensor(
                                            attn[:, hsl], attn[:, hsl],
                                            pat_sb[:, kc - 4 * qg, :],
                                            ALU.mult,
                                        )
                                    elif mask_mode == "general":
                                        nc.gpsimd.tensor_tensor(
                                            attn[:, hsl], attn[:, hsl],
                                            mtiles[kc], ALU.mult,
                                        )
                                for half in range(nhalf):
                                    kc = 2 * kk + half
                                    hl = 2 * pj + par
                                    nc.tensor.matmul(
                                        ctx[par][0:65, :],
                                        v_sb[:, kc, hl, :],
                                        attn[:, half * 512:(half + 1) * 512],
                                        start=(kc == 0),
                                        stop=(kc == kcmax - 1),
                                    )
                        # normalize both heads of the pair into ct_qg.
                        # One batched reciprocal; denominator rows live at
                        # partitions 0 and 32 (the only legal operand bases).
                        den = smalls.tile([33, 512], F32, tag="den")
                        nc.gpsimd.memset(den[:], 1.0)
                        for par in range(2):
                            nc.vector.tensor_copy(
                                den[32 * par:32 * par + 1, :],
                                ctx[par][64:65, :],
                            )
                        rec = smalls.tile([33, 512], MMD, tag="rec")
                        with nc.allow_low_precision(reason="f32r==fp32"):
                            nc.vector.reciprocal(rec[:], den[:])
                        recb = smalls.tile([1, 512], MMD, tag="recb")
                        nc.vector.tensor_copy(recb[:], rec[32:33, :])
                        rec_rows = (rec[0:1, :], recb[:])
                        for par in range(2):
                            bc_ps = bcps.tile([128, 512], F32, tag="bc")
                            nc.tensor.matmul(
                                bc_ps[0:64, :], ones_r[0:1, 0:64],
                                rec_rows[par],
                            )
                            bc_sb = smalls.tile([64, 512], F32, tag="bcs")
                            nc.vector.tensor_copy(bc_sb[:], bc_ps[0:64, :])
                            bp = par * 64
                            nc.vector.tensor_tensor(
                                ct_qg[bp:bp + 64, pj, :],
                                ctx[par][0:64, :], bc_sb[:], ALU.mult,
                            )

                    # ---- output projection for this q-group ----
                    for ns in range(4):
                        nt_i = qg * 4 + ns
                        nsl = slice(ns * 128, (ns + 1) * 128)
                        for oh in range(2):
                            po = bcps.tile([128, 512], F32, tag="bc")
                            for j in range(4):
                                nc.tensor.matmul(
                                    po[:],
                                    ct_qg[:, j, nsl],
                                    woT_sb[:, j, oh * 512:(oh + 1) * 512],
                                    start=(j == 0), stop=(j == 3),
                                )
                            ot = ostage.tile([128, 512], F32, tag="ot")
                            nc.vector.tensor_copy(ot[:], po[:])
                            nc.sync.dma_start(
                                pout_d[nt_i * 128:(nt_i + 1) * 128,
                                       oh * 512:(oh + 1) * 512],
                                ot[:],
                            )
    nc.compile()
    return nc


def _host_prepare(inputs):
    """Split the full problem into 8 per-core input maps + host-side info."""
    q = np.asarray(inputs["query"], dtype=np.float32)
    k = np.asarray(inputs["key"], dtype=np.float32)
    v = np.asarray(inputs["value"], dtype=np.float32)
    mask = np.asarray(inputs["mask"])
    w = {n: np.asarray(inputs[n], dtype=np.float32)
         for n in ("wq1", "wq2", "wk1", "wk2", "wv1", "wv2", "wo")}
    bias = {n: np.asarray(inputs[n], dtype=np.float32)
            for n in ("bq1", "bq2", "bk1", "bk2", "bv1", "bv2", "bo")}

    m = mask.reshape(S, S)
    if np.array_equal(m != 0, np.tril(np.ones((S, S), bool))):
        mask_mode = "causal"
    elif np.all(m != 0):
        mask_mode = "full"
    else:
        mask_mode = "general"

    pat = None
    m01T = None
    if mask_mode == "causal":
        kk = np.arange(128)[:, None]
        qq = np.arange(512)[None, :]
        pat = np.stack(
            [(kk + 128 * i <= qq).astype(np.float32) for i in range(4)], axis=1
        )  # [128, 4, 512]
        pat = np.ascontiguousarray(pat)
    elif mask_mode == "general":
        m01T = np.ascontiguousarray((m != 0).T.astype(np.float32))

    scale = 1.0 / np.sqrt(DK).astype(np.float32)

    if MM_DTYPE == "bf16":
        import ml_dtypes

        mmd_np = ml_dtypes.bfloat16
    else:
        mmd_np = np.float32

    def cvt(a):
        return np.ascontiguousarray(a).astype(mmd_np)

    in_maps = []
    for c in range(NCORES):
        b, g = divmod(c, 2)
        sl = slice(g * GCH, (g + 1) * GCH)
        im = {
            "xqT": cvt(q[b].T),
            "xkT": cvt(k[b].T),
            "xvT": cvt(v[b].T),
            "w1T_q": cvt(w["wq1"][sl].T),
            # fold the 1/sqrt(dk) score scale into the non-silu Q branch,
            # and 0.5 everywhere (silu computed as A*(1+tanh(A/2)) = 2*silu)
            "w2T_q": cvt(w["wq2"][sl].T * (scale * 0.5)),
            "w2T_k": cvt(w["wk2"][sl].T * 0.5),
            "w2T_v": cvt(w["wv2"][sl].T * 0.5),
            "w1T_k": cvt(w["wk1"][sl].T),
            "w1T_v": cvt(w["wv1"][sl].T),
            "b1_q": np.ascontiguousarray(bias["bq1"][sl].reshape(4, 128).T),
            "b1h_q": np.ascontiguousarray(
                (bias["bq1"][sl] * 0.5).reshape(4, 128).T),
            "b2_q": np.ascontiguousarray(
                (bias["bq2"][sl] * (scale * 0.5)).reshape(4, 128).T),
            "b1_k": np.ascontiguousarray(bias["bk1"][sl].reshape(4, 128).T),
            "b1h_k": np.ascontiguousarray(
                (bias["bk1"][sl] * 0.5).reshape(4, 128).T),
            "b2_k": np.ascontiguousarray(
                (bias["bk2"][sl] * 0.5).reshape(4, 128).T),
            "b1_v": cvt(bias["bv1"][sl].reshape(1, GCH)),
            "b2_v": cvt((bias["bv2"][sl] * 0.5).reshape(1, GCH)),
            "woT": cvt(
                w["wo"][:, sl].T.reshape(4, 128, D).transpose(1, 0, 2)),
        }
        if mask_mode == "causal":
            im["pat"] = cvt(pat)
        elif mask_mode == "general":
            im["m01T"] = cvt(m01T)
        in_maps.append(im)
    return mask_mode, in_maps, bias["bo"]


def kernel(**inputs):
    global LAST_RESULT
    mask_mode, in_maps, bo = _host_prepare(inputs)
    nc = build_program(mask_mode)

    import concourse.bass_utils as bu

    if TRACE:
        import types

        try:
            from trn_agent_boot.trn_boot import _ntff_profile_via_ctypes

            hook = _ntff_profile_via_ctypes("/opt/axon/libaxon_pjrt.so")
            m = types.ModuleType("antenv.axon_hooks")
            m.get_axon_ntff_profile_hook = lambda: hook
            import antenv  # noqa: F401

            sys.modules["antenv.axon_hooks"] = m
            bu.upload_artifacts = lambda d: "local://skipped"
        except Exception as e:
            print("profiling hook install failed:", e)

    res = bu.run_bass_kernel_spmd(
        nc, in_maps, core_ids=list(range(NCORES)),
        trace=TRACE, trace_cores=TRACE_CORES,
    )
    LAST_RESULT = res

    out = np.empty((B, S, D), dtype=np.float32)
    for b in range(B):
        out[b] = (res.results[2 * b]["pout"] + res.results[2 * b + 1]["pout"]
                  + bo[None, :])
    return out


# revision 14
# speedup vs baseline: 1.1811x; 1.0501x over previous
"""SwiGLU-projected causal MHA (B=4, S=2048, D=1024, H=16) on 8 TRN2 NeuronCores.

Sharding: core c -> (batch b = c//2, head-group g = c%2).  Each core computes
the SwiGLU Q/K/V projections for its 512 output channels (= 8 heads) of its
batch, runs causal attention for those heads, and produces a partial output
projection (contraction over its 512 channels).  The host sums the two
partials per batch and adds the output bias.

Device layout (per core):
  QT/KT [128p, 4j, 2048n]   channels on partitions (local ch = j*128 + p),
                            seq on free.  Head hl -> chunk hl//2, partition
                            base 64*(hl%2); head pairs run concurrently on
                            the PE via disjoint row groups (K=64 matmuls at
                            base 0 / base 64).
  V     [128p, 16nt, 8hl, 65]  seq on partitions (n = nt*128+p); per head 64
                            channels plus a ones column, so the AV matmul
                            emits the softmax denominator as output row 64.
  Scores are computed transposed, S^T [k-part, q-free], exp'd on the scalar
  engine straight out of PSUM (no max subtraction -- logits are bounded),
  masked multiplicatively on diagonal blocks only, and consumed as the
  moving operand of the AV matmul.  All matmuls run in float32r (full-rate
  fp32 streaming; measured ~1.6e-4 relative error on HW).
"""
import sys

sys.path.insert(0, "/opt/trn_rl_repo")
import numpy as np

import concourse.bacc as bacc
import concourse.tile as tile
import concourse.mybir as mybir

B, S, D = 4, 2048, 1024
H, DK = 16, 64
NCORES = 8
GCH = 512          # channels per core (8 heads)
NT = S // 128      # 16 seq chunks
F32 = mybir.dt.float32
F32R = mybir.dt.float32r
ACTF = mybir.ActivationFunctionType
ALU = mybir.AluOpType

TRACE = False          # set by test.py for profiling runs
TRACE_CORES = None
LAST_RESULT = None     # BassKernelResults stash for test.py
MM_DTYPE = "bf16"      # "bf16" (fast weight load) or "f32r" (higher precision)


def build_program(mask_mode):
    """mask_mode: 'causal' (tril), 'full' (all ones), 'general' (arbitrary)."""
    MMD = mybir.dt.bfloat16 if MM_DTYPE == "bf16" else F32R
    nc = bacc.Bacc("TRN2", target_bir_lowering=False, debug=False)

    xT = {s: nc.dram_tensor(f"x{s}T", [D, S], MMD, kind="ExternalInput")
          for s in "qkv"}
    w1T = {s: nc.dram_tensor(f"w1T_{s}", [D, GCH], MMD, kind="ExternalInput")
           for s in "qkv"}
    w2T = {s: nc.dram_tensor(f"w2T_{s}", [D, GCH], MMD, kind="ExternalInput")
           for s in "qkv"}
    bias_d = {}
    for s in "qk":
        for bn in ("b1", "b2", "b1h"):
            bias_d[f"{bn}_{s}"] = nc.dram_tensor(f"{bn}_{s}", [128, 4], F32,
                                                 kind="ExternalInput")
    b1v_d = nc.dram_tensor("b1_v", [1, GCH], MMD, kind="ExternalInput")
    b2v_d = nc.dram_tensor("b2_v", [1, GCH], MMD, kind="ExternalInput")
    woT_d = nc.dram_tensor("woT", [128, 4, D], MMD, kind="ExternalInput")
    pat_d = m01T_d = None
    if mask_mode == "causal":
        pat_d = nc.dram_tensor("pat", [128, 4, 512], MMD, kind="ExternalInput")
    elif mask_mode == "general":
        m01T_d = nc.dram_tensor("m01T", [S, S], MMD, kind="ExternalInput")
    pout_d = nc.dram_tensor("pout", [S, D], F32, kind="ExternalOutput")

    def kc_count(qg):
        return 4 * qg + 4 if mask_mode == "causal" else NT

    with tile.TileContext(nc) as tc:
        with (
            tc.tile_pool(name="persist", bufs=1) as persist,
        ):
            qt_sb = persist.tile([128, 4, S], MMD, tag="qt")
            kt_sb = persist.tile([128, 4, S], MMD, tag="kt")
            v_sb = persist.tile([128, NT, 8, 65], MMD, tag="v")
            woT_sb = persist.tile([128, 4, D], MMD, tag="wo")
            nc.sync.dma_start(woT_sb[:], woT_d[:])
            onesf = persist.tile([1, 128], F32, tag="onesf")
            ones_r = persist.tile([1, 128], MMD, tag="ones_r")
            nc.any.memset(onesf[:], 1.0)
            nc.vector.tensor_copy(ones_r[:], onesf[:])
            onescol = persist.tile([128, 1], F32, tag="onescol")
            nc.any.memset(onescol[:], 1.0)
            nc.vector.tensor_copy(
                v_sb[:, :, :, 64:65],
                onescol[:, None, :].to_broadcast([128, NT, 8, 1]),
            )
            if mask_mode == "causal":
                pat_sb = persist.tile([128, 4, 512], MMD, tag="pat")
                nc.sync.dma_start(pat_sb[:], pat_d[:])

            # ---------------- Phase A: SwiGLU projections ----------------
            with (
                tc.tile_pool(name="wpool", bufs=2) as wpool,
                tc.tile_pool(name="xpool", bufs=9) as xpool,
                tc.tile_pool(name="stage", bufs=4) as stage,
                tc.tile_pool(name="pps", bufs=6, space="PSUM") as pps,
            ):
                for s in "qkv":
                    w1sb = wpool.tile([128, 8, GCH], MMD, tag="w")
                    w2sb = wpool.tile([128, 8, GCH], MMD, tag="w")
                    nc.sync.dma_start(
                        w1sb[:], w1T[s][:].rearrange("(dc p) o -> p dc o", p=128)
                    )
                    nc.sync.dma_start(
                        w2sb[:], w2T[s][:].rearrange("(dc p) o -> p dc o", p=128)
                    )
                    if s != "v":
                        b1sb = persist.tile([128, 4], F32, tag=f"b1{s}")
                        b2sb = persist.tile([128, 4], F32, tag=f"b2{s}")
                        b1hsb = persist.tile([128, 4], F32, tag=f"b1h{s}")
                        nc.sync.dma_start(b1sb[:], bias_d[f"b1_{s}"][:])
                        nc.sync.dma_start(b2sb[:], bias_d[f"b2_{s}"][:])
                        nc.sync.dma_start(b1hsb[:], bias_d[f"b1h_{s}"][:])
                    else:
                        b1vr = persist.tile([1, GCH], MMD, tag="b1v")
                        b2vr = persist.tile([1, GCH], MMD, tag="b2v")
                        nc.sync.dma_start(b1vr[:], b1v_d[:])
                        nc.sync.dma_start(b2vr[:], b2v_d[:])

                    for t in range(4):  # 512-wide seq tiles
                        xts = []
                        for dc in range(8):
                            xt = xpool.tile([128, 512], MMD, tag="xt")
                            nc.sync.dma_start(
                                xt[:],
                                xT[s][dc * 128:(dc + 1) * 128,
                                      t * 512:(t + 1) * 512],
                            )
                            xts.append(xt)
                        for jh in range(2):
                            ps1 = [pps.tile([128, 512], F32, tag="pp",
                                            name=f"ps1_{i}")
                                   for i in range(2)]
                            ps2 = [pps.tile([128, 512], F32, tag="pp",
                                            name=f"ps2_{i}")
                                   for i in range(2)]
                            for dc in range(8):
                                for jj in range(2):
                                    j = jh * 2 + jj
                                    if s == "v":
                                        # seq on partitions: lhsT = x chunk
                                        nc.tensor.matmul(
                                            ps1[jj][:],
                                            xts[dc][:, j * 128:(j + 1) * 128],
                                            w1sb[:, dc, :],
                                            start=(dc == 0), stop=False,
                                        )
                                        nc.tensor.matmul(
                                            ps2[jj][:],
                                            xts[dc][:, j * 128:(j + 1) * 128],
                                            w2sb[:, dc, :],
                                            start=(dc == 0), stop=False,
                                        )
                                    else:
                                        # channels on partitions: lhsT = w chunk
                                        nc.tensor.matmul(
                                            ps1[jj][:],
                                            w1sb[:, dc, j * 128:(j + 1) * 128],
                                            xts[dc][:],
                                            start=(dc == 0), stop=(dc == 7),
                                        )
                                        nc.tensor.matmul(
                                            ps2[jj][:],
                                            w2sb[:, dc, j * 128:(j + 1) * 128],
                                            xts[dc][:],
                                            start=(dc == 0), stop=(dc == 7),
                                        )
                            for jj in range(2):
                                j = jh * 2 + jj
                                act = stage.tile([128, 512], F32, tag="act")
                                if s == "v":
                                    # fold the biases into the accumulation
                                    # (they vary along the free/channel dim)
                                    nc.tensor.matmul(
                                        ps1[jj][:], ones_r[:], b1vr[:],
                                        start=False, stop=True,
                                    )
                                    nc.tensor.matmul(
                                        ps2[jj][:], ones_r[:], b2vr[:],
                                        start=False, stop=True,
                                    )
                                    nc.scalar.activation(
                                        act[:], ps1[jj][:], ACTF.Tanh,
                                        scale=0.5,
                                    )
                                    u = stage.tile([128, 512], F32, tag="u")
                                    nc.vector.tensor_tensor(
                                        u[:], ps1[jj][:], act[:], ALU.mult
                                    )
                                    nc.vector.tensor_tensor(
                                        act[:], ps1[jj][:], u[:], ALU.add
                                    )
                                    nt_i = t * 4 + j
                                    nc.vector.tensor_tensor(
                                        v_sb[:, nt_i, :, 0:64],
                                        ps2[jj][:].rearrange(
                                            "p (h d) -> p h d", h=8
                                        ),
                                        act[:].rearrange(
                                            "p (h d) -> p h d", h=8
                                        ),
                                        ALU.mult,
                                    )
                                else:
                                    bias1 = b1sb[:, j:j + 1]
                                    bias2 = b2sb[:, j:j + 1]
                                    # act = tanh((A)/2), A = ps1 + b1
                                    nc.scalar.activation(
                                        act[:], ps1[jj][:], ACTF.Tanh,
                                        scale=0.5, bias=b1hsb[:, j:j + 1],
                                    )
                                    a_sb = stage.tile([128, 512], F32,
                                                      tag="u")
                                    nc.vector.tensor_scalar_add(
                                        a_sb[:], ps1[jj][:], bias1
                                    )
                                    # act = A*(1+tanh(A/2)) = 2*silu(A)
                                    nc.vector.scalar_tensor_tensor(
                                        act[:], act[:], 1.0, a_sb[:],
                                        op0=ALU.add, op1=ALU.mult,
                                    )
                                    dst = (qt_sb if s == "q" else kt_sb)[
                                        :, j, t * 512:(t + 1) * 512
                                    ]
                                    nc.vector.scalar_tensor_tensor(
                                        dst, ps2[jj][:], bias2, act[:],
                                        op0=ALU.add, op1=ALU.mult,
                                    )

            # ------------- Phase B+C: attention + output projection -------
            # Order all projection work (ACT: Silu) before attention work
            # (ACT: Exp) so the activation table set switches only once.
            tc.no_sync_barrier()
            with (
                tc.tile_pool(name="scps", bufs=4, space="PSUM") as scps,
                tc.tile_pool(name="cxps", bufs=2, space="PSUM") as cxps,
                tc.tile_pool(name="bcps", bufs=1, space="PSUM") as bcps,
                tc.tile_pool(name="apool", bufs=6) as apool,
                tc.tile_pool(name="ctpool", bufs=2) as ctpool,
                tc.tile_pool(name="smalls", bufs=4) as smalls,
                tc.tile_pool(name="ostage", bufs=4) as ostage,
                tc.tile_pool(name="mpool", bufs=2) as mpool,
            ):
                for qg in range(4):
                    kcmax = kc_count(qg)
                    qsl = slice(qg * 512, (qg + 1) * 512)
                    ct_qg = ctpool.tile([128, 4, 512], MMD, tag="ct")

                    mtiles = None
                    if mask_mode == "general":
                        mtiles = []
                        mt_sb = mpool.tile([128, NT, 512], MMD, tag="mt")
                        for kc in range(kcmax):
                            nc.sync.dma_start(
                                mt_sb[:, kc, :],
                                m01T_d[kc * 128:(kc + 1) * 128, qsl],
                            )
                            mtiles.append(mt_sb[:, kc, :])

                    for pj in range(4):   # head pair: hl = 2*pj (+1)
                        ctx = [cxps.tile([128, 512], F32, tag="cx",
                                         name=f"ctx_{i}")
                               for i in range(2)]
                        for kc in range(kcmax):
                            ksl = slice(kc * 128, (kc + 1) * 128)
                            sc = [scps.tile([128, 512], F32, tag="sc",
                                            name=f"sc_{i}")
                                  for i in range(2)]
                            for par in range(2):
                                bp = par * 64
                                nc.tensor.matmul(
                                    sc[par][:],
                                    kt_sb[bp:bp + 64, pj, ksl],
                                    qt_sb[bp:bp + 64, pj, qsl],
                                )
                            for par in range(2):
                                attn = apool.tile([128, 512], MMD, tag="at")
                                nc.scalar.activation(
                                    attn[:], sc[par][:], ACTF.Exp
                                )
                                if mask_mode == "causal" and kc >= 4 * qg:
                                    nc.vector.tensor_tensor(
                                        attn[:], attn[:],
                                        pat_sb[:, kc - 4 * qg, :],
                                        ALU.mult,
                                    )
                                elif mask_mode == "general":
                                    nc.vector.tensor_tensor(
                                        attn[:], attn[:],
                                        mtiles[kc], ALU.mult,
                                    )
                                hl = 2 * pj + par
                                nc.tensor.matmul(
                                    ctx[par][0:65, :],
                                    v_sb[:, kc, hl, :],
                                    attn[:],
                                    start=(kc == 0),
                                    stop=(kc == kcmax - 1),
                                )
                        # normalize both heads of the pair into ct_qg.
                        # One batched reciprocal; denominator rows live at
                        # partitions 0 and 32 (the only legal operand bases).
                        den = smalls.tile([33, 512], F32, tag="den")
                        nc.gpsimd.memset(den[:], 1.0)
                        for par in range(2):
                            nc.vector.tensor_copy(
                                den[32 * par:32 * par + 1, :],
                                ctx[par][64:65, :],
                            )
                        rec = smalls.tile([33, 512], MMD, tag="rec")
                        with nc.allow_low_precision(reason="f32r==fp32"):
                            nc.vector.reciprocal(rec[:], den[:])
                        recb = smalls.tile([1, 512], MMD, tag="recb")
                        nc.vector.tensor_copy(recb[:], rec[32:33, :])
                        rec_rows = (rec[0:1, :], recb[:])
                        for par in range(2):
                            bc_ps = bcps.tile([128, 512], F32, tag="bc")
                            nc.tensor.matmul(
                                bc_ps[0:64, :], ones_r[0:1, 0:64],
                                rec_rows[par],
                            )
                            bc_sb = smalls.tile([64, 512], F32, tag="bcs")
                            nc.vector.tensor_copy(bc_sb[:], bc_ps[0:64, :])
                            bp = par * 64
                            nc.vector.tensor_tensor(
                                ct_qg[bp:bp + 64, pj, :],
                                ctx[par][0:64, :], bc_sb[:], ALU.mult,
                            )

                    # ---- output projection for this q-group ----
                    for ns in range(4):
                        nt_i = qg * 4 + ns
                        nsl = slice(ns * 128, (ns + 1) * 128)
                        for oh in range(2):
                            po = bcps.tile([128, 512], F32, tag="bc")
                            for j in range(4):
                                nc.tensor.matmul(
                                    po[:],
                                    ct_qg[:, j, nsl],
                                    woT_sb[:, j, oh * 512:(oh + 1) * 512],
                                    start=(j == 0), stop=(j == 3),
                                )
                            ot = ostage.tile([128, 512], F32, tag="ot")
                            nc.vector.tensor_copy(ot[:], po[:])
                            nc.sync.dma_start(
                                pout_d[nt_i * 128:(nt_i + 1) * 128,
                                       oh * 512:(oh + 1) * 512],
                                ot[:],
                            )
    nc.compile()
    return nc


def _host_prepare(inputs):
    """Split the full problem into 8 per-core input maps + host-side info."""
    q = np.asarray(inputs["query"], dtype=np.float32)
    k = np.asarray(inputs["key"], dtype=np.float32)
    v = np.asarray(inputs["value"], dtype=np.float32)
    mask = np.asarray(inputs["mask"])
    w = {n: np.asarray(inputs[n], dtype=np.float32)
         for n in ("wq1", "wq2", "wk1", "wk2", "wv1", "wv2", "wo")}
    bias = {n: np.asarray(inputs[n], dtype=np.float32)
            for n in ("bq1", "bq2", "bk1", "bk2", "bv1", "bv2", "bo")}

    m = mask.reshape(S, S)
    if np.array_equal(m != 0, np.tril(np.ones((S, S), bool))):
        mask_mode = "causal"
    elif np.all(m != 0):
        mask_mode = "full"
    else:
        mask_mode = "general"

    pat = None
    m01T = None
    if mask_mode == "causal":
        kk = np.arange(128)[:, None]
        qq = np.arange(512)[None, :]
        pat = np.stack(
            [(kk + 128 * i <= qq).astype(np.float32) for i in range(4)], axis=1
        )  # [128, 4, 512]
        pat = np.ascontiguousarray(pat)
    elif mask_mode == "general":
        m01T = np.ascontiguousarray((m != 0).T.astype(np.float32))

    scale = 1.0 / np.sqrt(DK).astype(np.float32)

    if MM_DTYPE == "bf16":
        import ml_dtypes

        mmd_np = ml_dtypes.bfloat16
    else:
        mmd_np = np.float32

    def cvt(a):
        return np.ascontiguousarray(a).astype(mmd_np)

    in_maps = []
    for c in range(NCORES):
        b, g = divmod(c, 2)
        sl = slice(g * GCH, (g + 1) * GCH)
        im = {
            "xqT": cvt(q[b].T),
            "xkT": cvt(k[b].T),
            "xvT": cvt(v[b].T),
            "w1T_q": cvt(w["wq1"][sl].T),
            # fold the 1/sqrt(dk) score scale into the non-silu Q branch,
            # and 0.5 everywhere (silu computed as A*(1+tanh(A/2)) = 2*silu)
            "w2T_q": cvt(w["wq2"][sl].T * (scale * 0.5)),
            "w2T_k": cvt(w["wk2"][sl].T * 0.5),
            "w2T_v": cvt(w["wv2"][sl].T * 0.5),
            "w1T_k": cvt(w["wk1"][sl].T),
            "w1T_v": cvt(w["wv1"][sl].T),
            "b1_q": np.ascontiguousarray(bias["bq1"][sl].reshape(4, 128).T),
            "b1h_q": np.ascontiguousarray(
                (bias["bq1"][sl] * 0.5).reshape(4, 128).T),
            "b2_q": np.ascontiguousarray(
                (bias["bq2"][sl] * (scale * 0.5)).reshape(4, 128).T),
            "b1_k": np.ascontiguousarray(bias["bk1"][sl].reshape(4, 128).T),
            "b1h_k": np.ascontiguousarray(
                (bias["bk1"][sl] * 0.5).reshape(4, 128).T),
            "b2_k": np.ascontiguousarray(
                (bias["bk2"][sl] * 0.5).reshape(4, 128).T),
            "b1_v": cvt(bias["bv1"][sl].reshape(1, GCH)),
            "b2_v": cvt((bias["bv2"][sl] * 0.5).reshape(1, GCH)),
            "woT": cvt(
                w["wo"][:, sl].T.reshape(4, 128, D).transpose(1, 0, 2)),
        }
        if mask_mode == "causal":
            im["pat"] = cvt(pat)
        elif mask_mode == "general":
            im["m01T"] = cvt(m01T)
        in_maps.append(im)
    return mask_mode, in_maps, bias["bo"]


def kernel(**inputs):
    global LAST_RESULT
    mask_mode, in_maps, bo = _host_prepare(inputs)
    nc = build_program(mask_mode)

    import concourse.bass_utils as bu

    if TRACE:
        import types

        try:
            from trn_agent_boot.trn_boot import _ntff_profile_via_ctypes

            hook = _ntff_profile_via_ctypes("/opt/axon/libaxon_pjrt.so")
            m = types.ModuleType("antenv.axon_hooks")
            m.get_axon_ntff_profile_hook = lambda: hook
            import antenv  # noqa: F401

            sys.modules["antenv.axon_hooks"] = m
            bu.upload_artifacts = lambda d: "local://skipped"
        except Exception as e:
            print("profiling hook install failed:", e)

    res = bu.run_bass_kernel_spmd(
        nc, in_maps, core_ids=list(range(NCORES)),
        trace=TRACE, trace_cores=TRACE_CORES,
    )
    LAST_RESULT = res

    out = np.empty((B, S, D), dtype=np.float32)
    for b in range(B):
        out[b] = (res.results[2 * b]["pout"] + res.results[2 * b + 1]["pout"]
                  + bo[None, :])
    return out
